# revision 4
# baseline (speedup 1.0000x reference)
"""Trainium2 Bass kernel for Llama-style GQA attention (B=1, S=2048, D=4096,
H=32 q-heads, KVH=8 kv-heads, HD=128, rope theta=1e6, causal).

Sharding: tensor-parallel over heads across 8 NeuronCores. Core c owns q-heads
[4c, 4c+4) and kv-head c (wq/wk/wv column shards, fp16). Attention context is
AllGathered feature-sharded (fp16, per 512-token q-chunk) and the output
projection is column-parallel; the host concatenates the 8 fp32 column shards.

v2 design (single fused pipeline, PE-density-first):
  * Q/K are produced directly head-dim-major (weight chunk stationary, X^T
    moving, N=512) -- no PE transpose phase. RoPE runs head-major: the
    rotated half comes from an SBUF->SBUF DMA partition swap, then
    q' = q*cosT + swap(q)*sinT with host-built [128, S] tables.
  * V is produced head-major the same way, then PE-transposed per 128-token
    tile to the token-major layout PV needs (16 transposes total).
  * Attention exploits causality at 128-column granularity: diagonal k-tiles
    compute S/exp/PV only on the valid q-columns (N = 512-128r).
  * Softmax denominator: eager fp16 burst adds of the exp tiles, then one
    gpsimd partition_all_reduce (reduce+broadcast, no PSUM bank), DVE
    fast-reciprocal, and a single normalizing multiply per (head, chunk).
  * One TileContext region; emission interleaves QKV(qc), attention(qc-1) and
    out-projection(qc-2) so out-proj matmuls fill the PE while the exp chain
    runs. PSUM budget: qkv/vtrans 2 + scores 2 + ctx 2 + outproj 2 = 8 banks.
  * Out-projection per chunk runs as 2 passes x 2 PSUM accumulators over
    [128, 2, 256] AllGather-output tiles (same HBM bytes, half SBUF).
"""

import math

import numpy as np

import concourse.bass as bass
import concourse.bacc as bacc_mod
import concourse.mybir as mybir
import concourse.tile as tile
from concourse import bass_isa
from concourse.bass_utils import run_bass_kernel_spmd
from concourse.masks import make_identity

S = 2048
D = 4096
H = 32
KVH = 8
HD = 128
NC = 8
HPC = H // NC          # 4 q heads per core
QF = HPC * HD          # 512 q features per core
NT = S // 128          # 16 token tiles
KC = D // 128          # 32 contraction chunks
QCH = S // 512         # 4 q-chunks of 512
THETA = 1e6
SCALE = 1.0 / math.sqrt(HD)
F16 = mybir.dt.float16
F32 = mybir.dt.float32

_CACHE = {}


def build_program():
    nc = bacc_mod.Bacc(None, num_devices=NC)

    xt_d = nc.dram_tensor("xt", [D, S], F16, kind="ExternalInput")
    wq_d = nc.dram_tensor("wq", [D, QF], F16, kind="ExternalInput")
    wkv_d = nc.dram_tensor("wkv", [D, 256], F16, kind="ExternalInput")
    wo_d = nc.dram_tensor("wo", [D, QF], F16, kind="ExternalInput")
    cost_d = nc.dram_tensor("cost", [128, S], F16, kind="ExternalInput")
    sint_d = nc.dram_tensor("sint", [128, S], F16, kind="ExternalInput")
    kbias_d = nc.dram_tensor("kbias", [128, NT], F32, kind="ExternalInput")
    dmask_d = nc.dram_tensor("dmask", [128, 512], F16, kind="ExternalInput")
    out_d = nc.dram_tensor("out", [S, QF], F32, kind="ExternalOutput")

    cc_in = [nc.dram_tensor(f"cc_in{i}", [QF, 512], F16) for i in range(QCH)]
    cc_out = [nc.dram_tensor(f"cc_out{i}", [D, 512], F16, addr_space="Shared")
              for i in range(QCH)]

    with tile.TileContext(nc) as tc:
        with (
            tc.tile_pool(name="const", bufs=1) as const,
            tc.tile_pool(name="wpool", bufs=1) as wpool,
            tc.tile_pool(name="persist", bufs=1) as persist,
            tc.tile_pool(name="xtp", bufs=5) as xtp,
            tc.tile_pool(name="qtp", bufs=2) as qtp,
            tc.tile_pool(name="vtp", bufs=2) as vtp,
            tc.tile_pool(name="ropep", bufs=2) as ropep,
            tc.tile_pool(name="ptp", bufs=1) as ptp,
            tc.tile_pool(name="normp", bufs=2) as normp,
            tc.tile_pool(name="ctxp", bufs=2) as ctxp,
            tc.tile_pool(name="ccp", bufs=6) as ccp,
            tc.tile_pool(name="outp", bufs=2) as outp,
            tc.tile_pool(name="qkv_ps", bufs=2, space="PSUM") as qkv_ps,
            tc.tile_pool(name="s_ps", bufs=2, space="PSUM") as s_ps,
            tc.tile_pool(name="ctx_ps", bufs=2, space="PSUM") as ctx_ps,
            tc.tile_pool(name="o_ps", bufs=1, space="PSUM") as o_ps,
        ):
            # ---------------- constants / weights ----------------
            cosT = const.tile([128, S], F16, tag="cosT")
            sinT = const.tile([128, S], F16, tag="sinT")
            dmask = const.tile([128, 512], F16, tag="dmask")
            kbias = const.tile([128, NT], F32, tag="kbias")
            ident = const.tile([128, 128], F16, tag="ident")
            nc.gpsimd.dma_start(cosT[:], cost_d[:])
            nc.gpsimd.dma_start(sinT[:], sint_d[:])
            nc.gpsimd.dma_start(dmask[:], dmask_d[:])
            nc.gpsimd.dma_start(kbias[:], kbias_d[:])
            make_identity(nc, ident[:])

            wq = wpool.tile([128, KC, QF], F16, tag="wq")
            wkv = wpool.tile([128, KC, 256], F16, tag="wkv")
            wo = wpool.tile([128, KC, QF], F16, tag="wo")
            # wq first (needed by the very first pass), in 4 row-block chunks
            for b in range(4):
                nc.sync.dma_start(
                    wq[:, b * 8:(b + 1) * 8, :],
                    wq_d[b * 1024:(b + 1) * 1024, :].rearrange(
                        "(c p) f -> p c f", p=128),
                )
            kt = persist.tile([128, S], F16, tag="kt")        # [hd, tok]
            vv = persist.tile([128, NT, HD], F16, tag="vv")   # [tok%128, t, hd]

            # xt chunk blocks: [128, 8, 512] per (qc, blk)
            xtb = {}

            def load_xt(qc):
                for blk in range(4):
                    t = xtp.tile([128, 8, 512], F16, tag="xt", name=f"xt{qc}_{blk}")
                    nc.sync.dma_start(
                        t[:],
                        xt_d[blk * 1024:(blk + 1) * 1024,
                             qc * 512:(qc + 1) * 512].rearrange(
                                 "(c p) q -> p c q", p=128),
                    )
                    xtb[(qc, blk)] = t

            qt_tiles = {}
            ctxc_tiles = {}

            # ---------------- QKV pass (one head / K / V, one chunk) --------
            def qkv_pass(qc, kind, h=0):
                ps = qkv_ps.tile([128, 512], F32, tag="qkv", name=f"ps_{kind}{h}_{qc}")
                for kc in range(KC):
                    blk, j = kc // 8, kc % 8
                    rhs = xtb[(qc, blk)][:, j, :]
                    if kind == "q":
                        lhsT = wq[:, kc, h * 128:(h + 1) * 128]
                    elif kind == "k":
                        lhsT = wkv[:, kc, 0:128]
                    else:
                        lhsT = wkv[:, kc, 128:256]
                    nc.tensor.matmul(ps[:], lhsT, rhs,
                                     start=(kc == 0), stop=(kc == KC - 1))
                cols = slice(qc * 512, (qc + 1) * 512)
                if kind in ("q", "k"):
                    raw = ropep.tile([128, 512], F16, tag="raw", name="raw")
                    nc.scalar.copy(raw[:], ps[:])
                    t1 = ropep.tile([128, 512], F16, tag="t1", name="t1")
                    nc.vector.tensor_mul(t1[:], ps[:], cosT[:, cols])
                    sw = ropep.tile([128, 512], F16, tag="sw", name="sw")
                    nc.gpsimd.dma_start(sw[0:64, :], raw[64:128, :])
                    nc.gpsimd.dma_start(sw[64:128, :], raw[0:64, :])
                    t2 = ropep.tile([128, 512], F16, tag="t2", name="t2")
                    nc.vector.tensor_mul(t2[:], sw[:], sinT[:, cols])
                    if kind == "q":
                        qt_c = qt_tiles[qc]
                        nc.vector.tensor_add(qt_c[:, h, :], t1[:], t2[:])
                    else:
                        nc.vector.tensor_add(kt[:, cols], t1[:], t2[:])
                else:
                    vt = vtp.tile([128, 512], F16, tag="vt", name="vt")
                    nc.scalar.copy(vt[:], ps[:])
                    for ti in range(4):
                        tp = qkv_ps.tile([128, 128], F16, tag="qkv", name="tp")
                        nc.tensor.transpose(
                            tp[:], vt[:, ti * 128:(ti + 1) * 128], ident[:])
                        nc.vector.tensor_copy(vv[:, qc * 4 + ti, :], tp[:])

            def qkv_chunk_prologue(qc):
                load_xt(qc)
                qt_tiles[qc] = qtp.tile([128, HPC, 512], F16, tag="qt",
                                        name=f"qt{qc}")

            # ---------------- attention head ----------------
            def attn_head(qc, h):
                nkt = 4 * qc + 4
                qt_c = qt_tiles[qc]
                ctx = ctx_ps.tile([128, 512], F32, tag="ctx", name="ctx")
                ptsum = ptp.tile([128, 512], F16, tag="ptsum", name="ptsum",
                                 bufs=2)
                first_pt = []

                for ki in range(nkt):
                    r = ki - 4 * qc
                    n = 512 - 128 * r if r >= 0 else 512
                    off = 512 - n
                    sp = s_ps.tile([128, 512], F32, tag="sp", name="sp")
                    nc.tensor.matmul(
                        sp[:, :n],
                        kt[:, ki * 128:(ki + 1) * 128],
                        qt_c[:, h, off:512],
                        start=True, stop=True,
                    )
                    pt = ptp.tile([128, 512], F16, tag="pt", name="pt", bufs=8)
                    nc.scalar.activation(
                        pt[:, :n], sp[:, :n], mybir.ActivationFunctionType.Exp,
                        bias=kbias[:, ki:ki + 1], scale=SCALE,
                    )
                    if r >= 0:
                        ptm = ptp.tile([128, 512], F16, tag="ptm", name="ptm",
                                       bufs=6)
                        nc.vector.tensor_mul(ptm[:, :n], pt[:, :n],
                                             dmask[:, :n])
                        pt = ptm
                    nc.tensor.matmul(
                        ctx[:, off:512], vv[:, ki, :], pt[:, :n],
                        start=(ki == 0), stop=(ki == nkt - 1),
                    )
                    # eager denominator accumulation (fp16, pre-scaled 2^-8)
                    with nc.allow_low_precision(
                        reason="denoms pre-scaled 2^-8; fp16 sum ok"
                    ):
                        if ki == 0:
                            first_pt.append((pt, n, off))
                        elif ki == 1:
                            pt0, n0, off0 = first_pt.pop()
                            if off0 == 0 and off == 0:
                                nc.vector.tensor_add(ptsum[:], pt0[:, :512],
                                                     pt[:, :512])
                            else:
                                nc.vector.tensor_copy(ptsum[:, off0:512],
                                                      pt0[:, :n0])
                                nc.vector.tensor_add(ptsum[:, off:512],
                                                     ptsum[:, off:512],
                                                     pt[:, :n])
                        else:
                            nc.vector.tensor_add(ptsum[:, off:512],
                                                 ptsum[:, off:512], pt[:, :n])

                den = normp.tile([128, 512], F32, tag="den", name="den")
                nc.gpsimd.partition_all_reduce(den[:], ptsum[:], 128,
                                               bass_isa.ReduceOp.add)
                rec = normp.tile([128, 512], F32, tag="rec", name="rec")
                nc.vector.reciprocal_approx_fast(rec[:], den[:])
                nc.vector.tensor_mul(ctxc_tiles[qc][:, h, :], ctx[:], rec[:])

            def emit_allgather(qc):
                nc.gpsimd.dma_start(
                    cc_in[qc].rearrange("(h p) q -> p h q", p=128),
                    ctxc_tiles[qc][:],
                )
                nc.gpsimd.collective_compute(
                    "AllGather",
                    mybir.AluOpType.bypass,
                    replica_groups=[list(range(NC))],
                    ins=[cc_in[qc][:]],
                    outs=[cc_out[qc][:]],
                )

            def attn_chunk_prologue(qc):
                ctxc_tiles[qc] = ctxp.tile([128, HPC, 512], F16, tag="ctxc",
                                           name=f"ctxc{qc}")

            # ---------------- out-projection (one 256-token pass) ----------
            def outproj_pass(qc, p):
                o_acc = [o_ps.tile([128, QF], F32, tag=f"op{i}", name=f"op{i}",
                                   bufs=1) for i in range(2)]
                for fc2 in range(KC // 2):
                    cc_sb = ccp.tile([128, 2, 256], F16, tag="cc", name="cc")
                    nc.sync.dma_start(
                        cc_sb[:],
                        cc_out[qc][fc2 * 256:(fc2 + 1) * 256,
                                   p * 256:(p + 1) * 256].rearrange(
                                       "(c pp) q -> pp c q", pp=128),
                    )
                    for sub in range(2):
                        fc = fc2 * 2 + sub
                        for i in range(2):
                            nc.tensor.matmul(
                                o_acc[i][:],
                                cc_sb[:, sub, i * 128:(i + 1) * 128],
                                wo[:, fc, :],
                                start=(fc == 0), stop=(fc == KC - 1),
                            )
                for i in range(2):
                    t = qc * 4 + p * 2 + i
                    osb = outp.tile([128, QF], F32, tag="osb", name="osb")
                    if i == 0:
                        nc.vector.tensor_copy(osb[:], o_acc[i][:])
                    else:
                        nc.scalar.copy(osb[:], o_acc[i][:])
                    nc.sync.dma_start(out_d[t * 128:(t + 1) * 128, :], osb[:])

            # ---------------- pipeline emission ----------------
            qkv_chunk_prologue(0)
            # kv / out-proj weights are needed later than wq/xt(0); emit their
            # DMAs after the first chunk's loads so they don't delay the start
            for b in range(2):
                nc.sync.dma_start(
                    wkv[:, b * 16:(b + 1) * 16, :],
                    wkv_d[b * 2048:(b + 1) * 2048, :].rearrange(
                        "(c p) f -> p c f", p=128),
                )
            for b in range(4):
                nc.sync.dma_start(
                    wo[:, b * 8:(b + 1) * 8, :],
                    wo_d[b * 1024:(b + 1) * 1024, :].rearrange(
                        "(c p) f -> p c f", p=128),
                )
            for kind, h in (("q", 0), ("q", 1), ("q", 2), ("q", 3),
                            ("k", 0), ("v", 0)):
                qkv_pass(0, kind, h)

            for qc in range(1, QCH):
                qkv_chunk_prologue(qc)
                attn_chunk_prologue(qc - 1)
                qkv_pass(qc, "q", 0)
                attn_head(qc - 1, 0)
                qkv_pass(qc, "q", 1)
                attn_head(qc - 1, 1)
                if qc >= 2:
                    outproj_pass(qc - 2, 0)
                qkv_pass(qc, "q", 2)
                attn_head(qc - 1, 2)
                qkv_pass(qc, "q", 3)
                attn_head(qc - 1, 3)
                qkv_pass(qc, "k")
                emit_allgather(qc - 1)
                qkv_pass(qc, "v")
                if qc >= 2:
                    outproj_pass(qc - 2, 1)

            attn_chunk_prologue(3)
            attn_head(3, 0)
            outproj_pass(2, 0)
            attn_head(3, 1)
            attn_head(3, 2)
            outproj_pass(2, 1)
            attn_head(3, 3)
            emit_allgather(3)
            outproj_pass(3, 0)
            outproj_pass(3, 1)

    nc.compile()
    return nc


def _prep_inputs(hidden_states, attention_mask, position_ids, wq, wk, wv, wo):
    x = np.ascontiguousarray(np.asarray(hidden_states, np.float32)[0])     # [S, D]
    mask = np.asarray(attention_mask, np.float32)[0]
    pos = np.asarray(position_ids, np.int32)[0].astype(np.float32)
    wq = np.asarray(wq, np.float32)
    wk = np.asarray(wk, np.float32)
    wv = np.asarray(wv, np.float32)
    wo = np.asarray(wo, np.float32)

    f16 = np.float16
    xt = np.ascontiguousarray(x.T).astype(f16)                              # [D, S]

    freqs = 1.0 / THETA ** (np.arange(64, dtype=np.float32) / 64)
    t = pos[:, None] * freqs                                                # [S, 64]
    cos = np.cos(t).astype(np.float32)
    sin = np.sin(t).astype(np.float32)
    # head-dim-major tables [128, S]: row d -> cos(t * f_{d%64});
    # sin row d<64 gets the negative sign (q' = q*cos + swap(q)*sin)
    cosT = np.ascontiguousarray(np.concatenate([cos, cos], 1).T).astype(f16)
    sinT = np.ascontiguousarray(np.concatenate([-sin, sin], 1).T).astype(f16)

    # -8*ln2 shifts exp() by 2^-8 so fp16 P/denominator sums cannot overflow;
    # the shift cancels exactly in the softmax normalization.
    kbias = np.where(mask > 0, -8.0 * np.log(2.0), -1e30).astype(np.float32)
    kbias = np.ascontiguousarray(kbias.reshape(NT, 128).T)                  # [128, NT]

    # single diagonal causal mask (shift-invariant): keep iff f >= p
    dmask = (np.arange(512)[None, :] >= np.arange(128)[:, None]).astype(f16)

    in_maps = []
    for c in range(NC):
        in_maps.append({
            "xt": xt,
            "wq": np.ascontiguousarray(wq[:, c * QF:(c + 1) * QF]).astype(f16),
            "wkv": np.ascontiguousarray(
                np.concatenate([wk[:, c * HD:(c + 1) * HD],
                                wv[:, c * HD:(c + 1) * HD]], 1)).astype(f16),
            "wo": np.ascontiguousarray(wo[:, c * QF:(c + 1) * QF]).astype(f16),
            "cost": cosT, "sint": sinT,
            "kbias": kbias, "dmask": dmask,
        })
    return in_maps


def run(in_maps, trace=False):
    if "nc" not in _CACHE:
        _CACHE["nc"] = build_program()
    kwargs = {}
    if trace:
        kwargs = dict(trace=True, trace_cores=list(range(NC)))
    return run_bass_kernel_spmd(_CACHE["nc"], in_maps, list(range(NC)), **kwargs)


def kernel(hidden_states, attention_mask, position_ids, wq, wk, wv, wo):
    in_maps = _prep_inputs(hidden_states, attention_mask, position_ids,
                           wq, wk, wv, wo)
    res = run(in_maps, trace=False)
    shards = [res.results[c]["out"] for c in range(NC)]
    out = np.concatenate(shards, axis=1).astype(np.float32)                 # [S, D]
    return out[None]


# revision 10
# speedup vs baseline: 1.0556x; 1.0556x over previous
"""Trainium2 Bass kernel for Llama-style GQA attention (B=1, S=2048, D=4096,
H=32 q-heads, KVH=8 kv-heads, HD=128, rope theta=1e6, causal).

Sharding: tensor-parallel over heads across 8 NeuronCores. Core c owns q-heads
[4c, 4c+4) and kv-head c (wq/wk/wv column shards, fp16). Attention context is
AllGathered feature-sharded (fp16, per 512-token q-chunk) and the output
projection is column-parallel; the host concatenates the 8 fp32 column shards.

v2 design (single fused pipeline, PE-density-first):
  * Q/K are produced directly head-dim-major (weight chunk stationary, X^T
    moving, N=512) -- no PE transpose phase. RoPE runs head-major: the
    rotated half comes from an SBUF->SBUF DMA partition swap, then
    q' = q*cosT + swap(q)*sinT with host-built [128, S] tables.
  * V is produced head-major the same way, then PE-transposed per 128-token
    tile to the token-major layout PV needs (16 transposes total).
  * Attention exploits causality at 128-column granularity: diagonal k-tiles
    compute S/exp/PV only on the valid q-columns (N = 512-128r).
  * Softmax denominator: eager fp16 burst adds of the exp tiles, then one
    gpsimd partition_all_reduce (reduce+broadcast, no PSUM bank), DVE
    fast-reciprocal, and a single normalizing multiply per (head, chunk).
  * One TileContext region; emission interleaves QKV(qc), attention(qc-1) and
    out-projection(qc-2) so out-proj matmuls fill the PE while the exp chain
    runs. PSUM budget: qkv/vtrans 2 + scores 2 + ctx 2 + outproj 2 = 8 banks.
  * Out-projection per chunk runs as 2 passes x 2 PSUM accumulators over
    [128, 2, 256] AllGather-output tiles (same HBM bytes, half SBUF).
"""

import math

import numpy as np

import concourse.bass as bass
import concourse.bacc as bacc_mod
import concourse.mybir as mybir
import concourse.tile as tile
from concourse import bass_isa
from concourse.bass_utils import run_bass_kernel_spmd
from concourse.masks import make_identity

S = 2048
D = 4096
H = 32
KVH = 8
HD = 128
NC = 8
HPC = H // NC          # 4 q heads per core
QF = HPC * HD          # 512 q features per core
NT = S // 128          # 16 token tiles
KC = D // 128          # 32 contraction chunks
QCH = S // 512         # 4 q-chunks of 512
THETA = 1e6
SCALE = 1.0 / math.sqrt(HD)
F16 = mybir.dt.float16
F32 = mybir.dt.float32

_CACHE = {}


def build_program():
    nc = bacc_mod.Bacc(None, num_devices=NC)

    xt_d = nc.dram_tensor("xt", [D, S], F16, kind="ExternalInput")
    wq_d = nc.dram_tensor("wq", [D, QF], F16, kind="ExternalInput")
    wkv_d = nc.dram_tensor("wkv", [D, 256], F16, kind="ExternalInput")
    wo_d = nc.dram_tensor("wo", [D, QF], F16, kind="ExternalInput")
    cost_d = nc.dram_tensor("cost", [128, S], F16, kind="ExternalInput")
    sint_d = nc.dram_tensor("sint", [128, S], F16, kind="ExternalInput")
    kbias_d = nc.dram_tensor("kbias", [128, NT], F32, kind="ExternalInput")
    dmask_d = nc.dram_tensor("dmask", [128, 512], F16, kind="ExternalInput")
    out_d = nc.dram_tensor("out", [S, QF], F32, kind="ExternalOutput")

    cc_in = [nc.dram_tensor(f"cc_in{i}", [QF, 512], F16) for i in range(QCH)]
    cc_out = [nc.dram_tensor(f"cc_out{i}", [D, 512], F16, addr_space="Shared")
              for i in range(QCH)]

    with tile.TileContext(nc) as tc:
        with (
            tc.tile_pool(name="const", bufs=1) as const,
            tc.tile_pool(name="wpool", bufs=1) as wpool,
            tc.tile_pool(name="persist", bufs=1) as persist,
            tc.tile_pool(name="xtp", bufs=6) as xtp,
            tc.tile_pool(name="qtp", bufs=2) as qtp,
            tc.tile_pool(name="vtp", bufs=2) as vtp,
            tc.tile_pool(name="ropep", bufs=2) as ropep,
            tc.tile_pool(name="ptp", bufs=1) as ptp,
            tc.tile_pool(name="normp", bufs=2) as normp,
            tc.tile_pool(name="ctxp", bufs=2) as ctxp,
            tc.tile_pool(name="ccp", bufs=4) as ccp,
            tc.tile_pool(name="outp", bufs=2) as outp,
            tc.tile_pool(name="qkv_ps", bufs=2, space="PSUM") as qkv_ps,
            tc.tile_pool(name="s_ps", bufs=2, space="PSUM") as s_ps,
            tc.tile_pool(name="ctx_ps", bufs=2, space="PSUM") as ctx_ps,
            tc.tile_pool(name="o_ps", bufs=1, space="PSUM") as o_ps,
        ):
            # ---------------- constants / weights ----------------
            cosT = const.tile([128, S], F16, tag="cosT")
            sinT = const.tile([128, S], F16, tag="sinT")
            dmask = const.tile([128, 512], F16, tag="dmask")
            kbias = const.tile([128, NT], F32, tag="kbias")
            ident = const.tile([128, 128], F16, tag="ident")
            nc.gpsimd.dma_start(cosT[:], cost_d[:])
            nc.gpsimd.dma_start(sinT[:], sint_d[:])
            nc.gpsimd.dma_start(dmask[:], dmask_d[:])
            nc.gpsimd.dma_start(kbias[:], kbias_d[:])
            make_identity(nc, ident[:])

            wq = wpool.tile([128, KC, QF], F16, tag="wq")
            wkv = wpool.tile([128, KC, 256], F16, tag="wkv")
            wo = wpool.tile([128, KC, QF], F16, tag="wo")
            # wkv first (the k-pass leads each round), then wq; xt chunk 0
            # goes on the gpsimd queue so both HWDGE+SWDGE rings fill the
            # startup window in parallel
            for b in range(2):
                nc.sync.dma_start(
                    wkv[:, b * 16:(b + 1) * 16, :],
                    wkv_d[b * 2048:(b + 1) * 2048, :].rearrange(
                        "(c p) f -> p c f", p=128),
                )
            for b in range(4):
                nc.sync.dma_start(
                    wq[:, b * 8:(b + 1) * 8, :],
                    wq_d[b * 1024:(b + 1) * 1024, :].rearrange(
                        "(c p) f -> p c f", p=128),
                )
            kt = persist.tile([128, S], F16, tag="kt")        # [hd, tok]
            vv = persist.tile([128, NT, HD], F16, tag="vv")   # [tok%128, t, hd]

            # xt chunk blocks: [128, 8, 512] per (qc, blk)
            xtb = {}

            def load_xt(qc):
                eng = nc.gpsimd if qc == 0 else nc.sync
                for blk in range(4):
                    t = xtp.tile([128, 8, 512], F16, tag="xt", name=f"xt{qc}_{blk}")
                    eng.dma_start(
                        t[:],
                        xt_d[blk * 1024:(blk + 1) * 1024,
                             qc * 512:(qc + 1) * 512].rearrange(
                                 "(c p) q -> p c q", p=128),
                    )
                    xtb[(qc, blk)] = t

            qt_tiles = {}
            ctxc_tiles = {}

            # ---------------- QKV pass (one head / K / V, one chunk) --------
            def qkv_pass(qc, kind, h=0):
                ps = qkv_ps.tile([128, 512], F32, tag="qkv", name=f"ps_{kind}{h}_{qc}")
                for kc in range(KC):
                    blk, j = kc // 8, kc % 8
                    rhs = xtb[(qc, blk)][:, j, :]
                    if kind == "q":
                        lhsT = wq[:, kc, h * 128:(h + 1) * 128]
                    elif kind == "k":
                        lhsT = wkv[:, kc, 0:128]
                    else:
                        lhsT = wkv[:, kc, 128:256]
                    nc.tensor.matmul(ps[:], lhsT, rhs,
                                     start=(kc == 0), stop=(kc == KC - 1))
                cols = slice(qc * 512, (qc + 1) * 512)
                if kind in ("q", "k"):
                    raw = ropep.tile([128, 512], F16, tag="raw", name="raw")
                    nc.scalar.copy(raw[:], ps[:])
                    t1 = ropep.tile([128, 512], F16, tag="t1", name="t1")
                    nc.vector.tensor_mul(t1[:], ps[:], cosT[:, cols])
                    sw = ropep.tile([128, 512], F16, tag="sw", name="sw")
                    nc.gpsimd.dma_start(sw[0:64, :], raw[64:128, :])
                    nc.gpsimd.dma_start(sw[64:128, :], raw[0:64, :])
                    t2 = ropep.tile([128, 512], F16, tag="t2", name="t2")
                    nc.vector.tensor_mul(t2[:], sw[:], sinT[:, cols])
                    if kind == "q":
                        qt_c = qt_tiles[qc]
                        nc.vector.tensor_add(qt_c[:, h, :], t1[:], t2[:])
                    else:
                        nc.vector.tensor_add(kt[:, cols], t1[:], t2[:])
                else:
                    vt = vtp.tile([128, 512], F16, tag="vt", name="vt")
                    nc.scalar.copy(vt[:], ps[:])
                    for ti in range(4):
                        tp = qkv_ps.tile([128, 128], F16, tag="qkv", name="tp")
                        nc.tensor.transpose(
                            tp[:], vt[:, ti * 128:(ti + 1) * 128], ident[:])
                        nc.vector.tensor_copy(vv[:, qc * 4 + ti, :], tp[:])

            def qkv_chunk_prologue(qc):
                load_xt(qc)
                qt_tiles[qc] = qtp.tile([128, HPC, 512], F16, tag="qt",
                                        name=f"qt{qc}")

            # ---------------- attention head ----------------
            def attn_head(qc, h):
                nkt = 4 * qc + 4
                qt_c = qt_tiles[qc]
                ctx = ctx_ps.tile([128, 512], F32, tag="ctx", name="ctx")
                ptsum = ptp.tile([128, 512], F16, tag="ptsum", name="ptsum",
                                 bufs=2)
                first_pt = []

                for ki in range(nkt):
                    r = ki - 4 * qc
                    n = 512 - 128 * r if r >= 0 else 512
                    off = 512 - n
                    sp = s_ps.tile([128, 512], F32, tag="sp", name="sp")
                    nc.tensor.matmul(
                        sp[:, :n],
                        kt[:, ki * 128:(ki + 1) * 128],
                        qt_c[:, h, off:512],
                        start=True, stop=True,
                    )
                    pt = ptp.tile([128, 512], F16, tag="pt", name="pt", bufs=8)
                    nc.scalar.activation(
                        pt[:, :n], sp[:, :n], mybir.ActivationFunctionType.Exp,
                        bias=kbias[:, ki:ki + 1], scale=SCALE,
                    )
                    if r >= 0:
                        ptm = ptp.tile([128, 512], F16, tag="ptm", name="ptm",
                                       bufs=6)
                        nc.vector.tensor_mul(ptm[:, :n], pt[:, :n],
                                             dmask[:, :n])
                        pt = ptm
                    nc.tensor.matmul(
                        ctx[:, off:512], vv[:, ki, :], pt[:, :n],
                        start=(ki == 0), stop=(ki == nkt - 1),
                    )
                    # eager denominator accumulation (fp16, pre-scaled 2^-8)
                    with nc.allow_low_precision(
                        reason="denoms pre-scaled 2^-8; fp16 sum ok"
                    ):
                        if ki == 0:
                            first_pt.append((pt, n, off))
                        elif ki == 1:
                            pt0, n0, off0 = first_pt.pop()
                            if off0 == 0 and off == 0:
                                nc.vector.tensor_add(ptsum[:], pt0[:, :512],
                                                     pt[:, :512])
                            else:
                                nc.vector.tensor_copy(ptsum[:, off0:512],
                                                      pt0[:, :n0])
                                nc.vector.tensor_add(ptsum[:, off:512],
                                                     ptsum[:, off:512],
                                                     pt[:, :n])
                        else:
                            nc.vector.tensor_add(ptsum[:, off:512],
                                                 ptsum[:, off:512], pt[:, :n])

                den = normp.tile([128, 512], F32, tag="den", name="den")
                nc.gpsimd.partition_all_reduce(den[:], ptsum[:], 128,
                                               bass_isa.ReduceOp.add)
                rec = normp.tile([128, 512], F32, tag="rec", name="rec")
                nc.vector.reciprocal_approx_fast(rec[:], den[:])
                nc.vector.tensor_mul(ctxc_tiles[qc][:, h, :], ctx[:], rec[:])

            def emit_allgather(qc):
                nc.gpsimd.dma_start(
                    cc_in[qc].rearrange("(h p) q -> p h q", p=128),
                    ctxc_tiles[qc][:],
                )
                nc.gpsimd.collective_compute(
                    "AllGather",
                    mybir.AluOpType.bypass,
                    replica_groups=[list(range(NC))],
                    ins=[cc_in[qc][:]],
                    outs=[cc_out[qc][:]],
                )

            def attn_chunk_prologue(qc):
                ctxc_tiles[qc] = ctxp.tile([128, HPC, 512], F16, tag="ctxc",
                                           name=f"ctxc{qc}")

            # ---------------- out-projection (one 256-token pass) ----------
            def outproj_pass(qc, p):
                o_acc = [o_ps.tile([128, QF], F32, tag=f"op{i}", name=f"op{i}",
                                   bufs=1) for i in range(2)]
                for fc2 in range(KC // 2):
                    cc_sb = ccp.tile([128, 2, 256], F16, tag="cc", name="cc")
                    nc.sync.dma_start(
                        cc_sb[:],
                        cc_out[qc][fc2 * 256:(fc2 + 1) * 256,
                                   p * 256:(p + 1) * 256].rearrange(
                                       "(c pp) q -> pp c q", pp=128),
                    )
                    for sub in range(2):
                        fc = fc2 * 2 + sub
                        for i in range(2):
                            nc.tensor.matmul(
                                o_acc[i][:],
                                cc_sb[:, sub, i * 128:(i + 1) * 128],
                                wo[:, fc, :],
                                start=(fc == 0), stop=(fc == KC - 1),
                            )
                for i in range(2):
                    t = qc * 4 + p * 2 + i
                    osb = outp.tile([128, QF], F32, tag="osb", name="osb")
                    if i == 0:
                        nc.vector.tensor_copy(osb[:], o_acc[i][:])
                    else:
                        nc.scalar.copy(osb[:], o_acc[i][:])
                    nc.sync.dma_start(out_d[t * 128:(t + 1) * 128, :], osb[:])

            # ---------------- pipeline emission ----------------
            # out-proj weights are needed only from round 1 on; emit after the
            # first chunk's critical loads
            for b in range(4):
                nc.sync.dma_start(
                    wo[:, b * 8:(b + 1) * 8, :],
                    wo_d[b * 1024:(b + 1) * 1024, :].rearrange(
                        "(c p) f -> p c f", p=128),
                )
            # round qc: K/V first (attention qc needs them), Q heads with
            # attention trailing one head behind, AllGather at round end, and
            # the PREVIOUS chunk's out-projection as PE filler material.
            for qc in range(QCH):
                qkv_chunk_prologue(qc)
                attn_chunk_prologue(qc)
                qkv_pass(qc, "k")
                qkv_pass(qc, "v")
                qkv_pass(qc, "q", 0)
                qkv_pass(qc, "q", 1)
                attn_head(qc, 0)
                qkv_pass(qc, "q", 2)
                attn_head(qc, 1)
                qkv_pass(qc, "q", 3)
                attn_head(qc, 2)
                attn_head(qc, 3)
                emit_allgather(qc)
                if qc >= 1:
                    outproj_pass(qc - 1, 0)
                    outproj_pass(qc - 1, 1)

            outproj_pass(3, 0)
            outproj_pass(3, 1)

    nc.compile()
    return nc


def _prep_inputs(hidden_states, attention_mask, position_ids, wq, wk, wv, wo):
    x = np.ascontiguousarray(np.asarray(hidden_states, np.float32)[0])     # [S, D]
    mask = np.asarray(attention_mask, np.float32)[0]
    pos = np.asarray(position_ids, np.int32)[0].astype(np.float32)
    wq = np.asarray(wq, np.float32)
    wk = np.asarray(wk, np.float32)
    wv = np.asarray(wv, np.float32)
    wo = np.asarray(wo, np.float32)

    f16 = np.float16
    xt = np.ascontiguousarray(x.T).astype(f16)                              # [D, S]

    freqs = 1.0 / THETA ** (np.arange(64, dtype=np.float32) / 64)
    t = pos[:, None] * freqs                                                # [S, 64]
    cos = np.cos(t).astype(np.float32)
    sin = np.sin(t).astype(np.float32)
    # head-dim-major tables [128, S]: row d -> cos(t * f_{d%64});
    # sin row d<64 gets the negative sign (q' = q*cos + swap(q)*sin)
    cosT = np.ascontiguousarray(np.concatenate([cos, cos], 1).T).astype(f16)
    sinT = np.ascontiguousarray(np.concatenate([-sin, sin], 1).T).astype(f16)

    # -8*ln2 shifts exp() by 2^-8 so fp16 P/denominator sums cannot overflow;
    # the shift cancels exactly in the softmax normalization.
    kbias = np.where(mask > 0, -8.0 * np.log(2.0), -1e30).astype(np.float32)
    kbias = np.ascontiguousarray(kbias.reshape(NT, 128).T)                  # [128, NT]

    # single diagonal causal mask (shift-invariant): keep iff f >= p
    dmask = (np.arange(512)[None, :] >= np.arange(128)[:, None]).astype(f16)

    in_maps = []
    for c in range(NC):
        in_maps.append({
            "xt": xt,
            "wq": np.ascontiguousarray(wq[:, c * QF:(c + 1) * QF]).astype(f16),
            "wkv": np.ascontiguousarray(
                np.concatenate([wk[:, c * HD:(c + 1) * HD],
                                wv[:, c * HD:(c + 1) * HD]], 1)).astype(f16),
            "wo": np.ascontiguousarray(wo[:, c * QF:(c + 1) * QF]).astype(f16),
            "cost": cosT, "sint": sinT,
            "kbias": kbias, "dmask": dmask,
        })
    return in_maps


def run(in_maps, trace=False):
    if "nc" not in _CACHE:
        _CACHE["nc"] = build_program()
    kwargs = {}
    if trace:
        kwargs = dict(trace=True, trace_cores=list(range(NC)))
    return run_bass_kernel_spmd(_CACHE["nc"], in_maps, list(range(NC)), **kwargs)


def kernel(hidden_states, attention_mask, position_ids, wq, wk, wv, wo):
    in_maps = _prep_inputs(hidden_states, attention_mask, position_ids,
                           wq, wk, wv, wo)
    res = run(in_maps, trace=False)
    shards = [res.results[c]["out"] for c in range(NC)]
    out = np.concatenate(shards, axis=1).astype(np.float32)                 # [S, D]
    return out[None]
